# revision 2
# baseline (speedup 1.0000x reference)
"""Dual-key additive attention (nn_Attention_58059367908033) on 8 trn2 NeuronCores.

Reference computation (per batch b, head h, with n = 64*64 = 4096 positions,
d = 128, scale = d**-0.5):
    q  = Wq_h  @ fmap[b]          # [d, n]   (channels-major, "n" = spatial)
    k1 = Wk1_h @ fmap[b]          # [d, n]
    v  = Wv_h  @ fmap[b]          # [d, n]
    k2 = Wk2_h @ x[b]             # [d, n]
    sim  = (scale * q)^T (k1+k2)  # [n, n]  (q rows, key cols)
    attn = softmax(sim, axis=-1)
    out[b, h*d:(h+1)*d] = (attn @ v^T)^T  # [d, n] -> reshape [d, 64, 64]

Sharding: 8 cores = (b in 2) x (h in 2) x (key-half kh in 2).  Each core
computes, for its (b, h) and its 2048-key slice, the *unnormalized*
    U[d, q]   = sum_{k in slice} exp(scale * sim[k, q]) * vT[k, d]
    D[1, q]   = sum_{k in slice} exp(scale * sim[k, q])
streamed flash-attention style (no max subtraction: |scale*sim| is O(1) for
these inputs, fp32 exp is exact-safe).  The host adds the two key-half
partials and divides -- mathematically exact softmax-attention.

On-chip layout is fully transposed (keys on partitions for exp, contraction
over d for QK^T and over k for PV), so no transposes are needed anywhere and
U comes out channels-major, matching the output layout directly.
"""

import numpy as np

import concourse.bass as bass
import concourse.mybir as mybir
import concourse.tile as tile
from concourse import bacc
from concourse.bass_utils import run_bass_kernel_spmd

HEADS = 2
D = 128          # dim head
C1 = 256         # fmap channels
C2 = 2048        # x channels
N = 4096         # spatial positions (64*64) = queries = keys
KSL = 2048       # keys per core (half)
SCALE = float(D) ** -0.5

F32 = mybir.dt.float32

# key-chunk = 512 keys (4 k-tiles of 128); query-chunk = 512 queries
KC = 4           # key chunks per core
KT = 4           # k-tiles (128) per key chunk
QC = 8           # query chunks of 512
QW = 512

_COMPILED = {}


def _build_program():
    nc = bacc.Bacc("TRN2", target_bir_lowering=False, debug=False, num_devices=8)

    # ---- DRAM parameters (per-core data, same program on all 8 cores) ----
    d_fmap = nc.dram_tensor("fmap_b", [2, 128, N], F32, kind="ExternalInput").ap()
    d_fmapk = nc.dram_tensor("fmap_k", [2, 128, KSL], F32, kind="ExternalInput").ap()
    d_xs = nc.dram_tensor("xs", [16, 128, KSL], F32, kind="ExternalInput").ap()
    d_wqT = nc.dram_tensor("wqT", [2, 128, D], F32, kind="ExternalInput").ap()
    d_wk1T = nc.dram_tensor("wk1T", [2, 128, D], F32, kind="ExternalInput").ap()
    d_wvT = nc.dram_tensor("wvT", [2, 128, D], F32, kind="ExternalInput").ap()
    d_wk2T = nc.dram_tensor("wk2T", [16, 128, D], F32, kind="ExternalInput").ap()
    d_outU = nc.dram_tensor("outU", [128, N], F32, kind="ExternalOutput").ap()
    d_den = nc.dram_tensor("denom", [1, N], F32, kind="ExternalOutput").ap()

    with tile.TileContext(nc) as tc:
        with (
            tc.tile_pool(name="wts", bufs=1) as wts,
            tc.tile_pool(name="fm", bufs=1) as fm,
            tc.tile_pool(name="big", bufs=1) as big,
            tc.tile_pool(name="xs", bufs=2) as xsp,
            tc.tile_pool(name="ex", bufs=3) as exp_pool,
            tc.tile_pool(name="st", bufs=2) as st,
            tc.tile_pool(name="ps_k", bufs=1, space="PSUM") as ps_k,
            tc.tile_pool(name="ps_s", bufs=3, space="PSUM") as ps_s,
            tc.tile_pool(name="ps_o", bufs=2, space="PSUM") as ps_o,
            tc.tile_pool(name="ps_d", bufs=2, space="PSUM") as ps_d,
        ):
            # ---- load weights + fmap ----
            wqT = [wts.tile([128, D], F32, tag=f"wqT{t}", name=f"wqT{t}") for t in range(2)]
            wk1T = [wts.tile([128, D], F32, tag=f"wk1T{t}", name=f"wk1T{t}") for t in range(2)]
            wvT = [wts.tile([128, D], F32, tag=f"wvT{t}", name=f"wvT{t}") for t in range(2)]
            wk2T = [wts.tile([128, D], F32, tag=f"wk2T{t}", name=f"wk2T{t}") for t in range(16)]
            for t in range(2):
                nc.sync.dma_start(wqT[t][:], d_wqT[t])
                nc.sync.dma_start(wk1T[t][:], d_wk1T[t])
                nc.sync.dma_start(wvT[t][:], d_wvT[t])
            for t in range(16):
                nc.sync.dma_start(wk2T[t][:], d_wk2T[t])

            fmap = [fm.tile([128, N], F32, tag=f"fmap{t}", name=f"fmap{t}") for t in range(2)]
            fmapk = [fm.tile([128, KSL], F32, tag=f"fmapk{t}", name=f"fmapk{t}") for t in range(2)]
            for t in range(2):
                nc.sync.dma_start(fmap[t][:], d_fmap[t])
                nc.sync.dma_start(fmapk[t][:], d_fmapk[t])

            ones = wts.tile([128, 1], F32, tag="ones")
            nc.vector.memset(ones[:], 1.0)

            # x chunk tiles: per (chunk parity, ct) slots via bufs=2 on tag ct
            def load_x_chunk(kc):
                tiles = []
                for ct in range(16):
                    xt = xsp.tile([128, QW], F32, tag=f"x{ct}", name=f"x{ct}")
                    nc.sync.dma_start(xt[:], d_xs[ct][:, kc * QW:(kc + 1) * QW])
                    tiles.append(xt)
                return tiles

            x_tiles = load_x_chunk(0)

            # ---- q = Wq @ fmap  -> q_sb [d=128, N] ----
            q_sb = big.tile([128, N], F32, tag="q")
            for nch in range(8):
                ps = ps_s.tile([128, QW], F32, tag="ps_sim")
                sl = slice(nch * QW, (nch + 1) * QW)
                nc.tensor.matmul(ps[:], wqT[0][:], fmap[0][:, sl], start=True, stop=False)
                nc.tensor.matmul(ps[:], wqT[1][:], fmap[1][:, sl], start=False, stop=True)
                nc.vector.tensor_copy(q_sb[:, sl], ps[:])

            # ---- vT tiles [k=128, d] via fmap_k-stationary matmuls ----
            vT = big.tile([128, 16, D], F32, tag="vT")
            for kt in range(16):
                ps = ps_s.tile([128, D], F32, tag="ps_sim")
                ksl = slice(kt * 128, (kt + 1) * 128)
                nc.tensor.matmul(ps[:], fmapk[0][:, ksl], wvT[0][:], start=True, stop=False)
                nc.tensor.matmul(ps[:], fmapk[1][:, ksl], wvT[1][:], start=False, stop=True)
                nc.scalar.copy(vT[:, kt, :], ps[:])

            ksum = big.tile([128, KSL], F32, tag="ksum")
            outU = big.tile([128, N], F32, tag="outU")
            den = big.tile([1, N], F32, tag="den")

            # ---- main loop over key chunks ----
            for kc in range(KC):
                # ksum[:, kc] = Wk1 @ fmap_k[:, kc] + Wk2 @ xs[:, kc]
                kps = ps_k.tile([128, QW], F32, tag="ps_ksum")
                sl = slice(kc * QW, (kc + 1) * QW)
                nc.tensor.matmul(kps[:], wk1T[0][:], fmapk[0][:, sl], start=True, stop=False)
                nc.tensor.matmul(kps[:], wk1T[1][:], fmapk[1][:, sl], start=False, stop=False)
                for ct in range(16):
                    nc.tensor.matmul(kps[:], wk2T[ct][:], x_tiles[ct][:],
                                     start=False, stop=(ct == 15))
                nc.vector.tensor_copy(ksum[:, sl], kps[:])

                if kc + 1 < KC:
                    x_tiles = load_x_chunk(kc + 1)

                # attention over this chunk's 4 k-tiles, all 8 query chunks
                for qc in range(QC):
                    qsl = slice(qc * QW, (qc + 1) * QW)
                    ops = ps_o.tile([128, QW], F32, tag="ps_out")
                    dps = ps_d.tile([1, QW], F32, tag="ps_den")
                    for kt in range(KT):
                        kk = kc * KT + kt
                        sps = ps_s.tile([128, QW], F32, tag="ps_sim")
                        nc.tensor.matmul(
                            sps[:], ksum[:, kk * 128:(kk + 1) * 128], q_sb[:, qsl],
                            start=True, stop=True)
                        et = exp_pool.tile([128, QW], F32, tag="exp")
                        nc.scalar.activation(et[:], sps[:],
                                             mybir.ActivationFunctionType.Exp,
                                             scale=SCALE)
                        nc.tensor.matmul(ops[:], vT[:, kk, :], et[:],
                                         start=(kt == 0), stop=(kt == KT - 1))
                        nc.tensor.matmul(dps[:], ones[:], et[:],
                                         start=(kt == 0), stop=(kt == KT - 1))
                    if kc == 0:
                        nc.vector.tensor_copy(outU[:, qsl], ops[:])
                        nc.vector.tensor_copy(den[:, qsl], dps[:])
                    else:
                        nc.vector.tensor_add(outU[:, qsl], outU[:, qsl], ops[:])
                        nc.vector.tensor_add(den[:, qsl], den[:, qsl], dps[:])
                        if kc == KC - 1:
                            st_t = st.tile([128, QW], F32, tag="st")
                            nc.vector.tensor_copy(st_t[:], outU[:, qsl])
                            nc.sync.dma_start(d_outU[:, qsl], st_t[:])
            nc.sync.dma_start(d_den[:], den[:])

    nc.compile()
    return nc


def _prep_inputs(fmap, x, Wqkv, Wk2):
    """Host-side slicing: per-core input dicts. Core c = b*4 + h*2 + kh."""
    fmap = np.ascontiguousarray(fmap, dtype=np.float32)
    x = np.ascontiguousarray(x, dtype=np.float32)
    Wqkv = np.ascontiguousarray(Wqkv, dtype=np.float32)
    Wk2 = np.ascontiguousarray(Wk2, dtype=np.float32)

    in_maps = []
    for c in range(8):
        b, h, kh = c // 4, (c // 2) % 2, c % 2
        fb = fmap[b].reshape(C1, N)
        xb = x[b].reshape(C2, N)
        ks = slice(kh * KSL, (kh + 1) * KSL)
        wq = Wqkv[h * D:(h + 1) * D]              # [128, 256]
        wk1 = Wqkv[C1 + h * D:C1 + (h + 1) * D]
        wv = Wqkv[2 * C1 + h * D:2 * C1 + (h + 1) * D]
        wk2 = Wk2[h * D:(h + 1) * D]              # [128, 2048]
        in_maps.append({
            "fmap_b": np.ascontiguousarray(fb.reshape(2, 128, N)),
            "fmap_k": np.ascontiguousarray(fb[:, ks].reshape(2, 128, KSL)),
            "xs": np.ascontiguousarray(xb[:, ks].reshape(16, 128, KSL)),
            "wqT": np.ascontiguousarray(wq.T.reshape(2, 128, D)),
            "wk1T": np.ascontiguousarray(wk1.T.reshape(2, 128, D)),
            "wvT": np.ascontiguousarray(wv.T.reshape(2, 128, D)),
            "wk2T": np.ascontiguousarray(wk2.T.reshape(16, 128, D)),
        })
    return in_maps


def _combine(results):
    """Host epilogue: add key-half partials, normalize, assemble output."""
    out = np.empty((2, HEADS * D, 64, 64), dtype=np.float32)
    for b in range(2):
        for h in range(2):
            c0 = b * 4 + h * 2
            U = results[c0]["outU"] + results[c0 + 1]["outU"]     # [128, N]
            Dn = results[c0]["denom"] + results[c0 + 1]["denom"]  # [1, N]
            out[b, h * D:(h + 1) * D] = (U / Dn).reshape(D, 64, 64)
    return out


def run_on_device(in_maps, trace=False, **kw):
    if "nc" not in _COMPILED:
        _COMPILED["nc"] = _build_program()
    return run_bass_kernel_spmd(_COMPILED["nc"], in_maps, list(range(8)),
                                trace=trace, **kw)


def kernel(fmap, x, Wqkv, Wk2):
    in_maps = _prep_inputs(fmap, x, Wqkv, Wk2)
    res = run_on_device(in_maps)
    return _combine(res.results)


# revision 3
# speedup vs baseline: 2.9403x; 2.9403x over previous
"""Dual-key additive attention (nn_Attention_58059367908033) on 8 trn2 NeuronCores.

Reference computation (per batch b, head h, with n = 64*64 = 4096 positions,
d = 128, scale = d**-0.5):
    q  = Wq_h  @ fmap[b]          # [d, n]   (channels-major, "n" = spatial)
    k1 = Wk1_h @ fmap[b]          # [d, n]
    v  = Wv_h  @ fmap[b]          # [d, n]
    k2 = Wk2_h @ x[b]             # [d, n]
    sim  = (scale * q)^T (k1+k2)  # [n, n]  (q rows, key cols)
    attn = softmax(sim, axis=-1)
    out[b, h*d:(h+1)*d] = (attn @ v^T)^T  # [d, n] -> reshape [d, 64, 64]

Sharding: 8 cores = (b in 2) x (h in 2) x (key-half kh in 2).  Each core
computes, for its (b, h) and its 2048-key slice, the *unnormalized*
    U[d, q]   = sum_{k in slice} exp(scale * sim[k, q]) * vT[k, d]
    D[1, q]   = sum_{k in slice} exp(scale * sim[k, q])
streamed flash-attention style (no max subtraction: |scale*sim| is O(1) for
these inputs, fp32 exp is exact-safe).  The host adds the two key-half
partials and divides -- mathematically exact softmax-attention.

On-chip layout is fully transposed (keys on partitions for exp, contraction
over d for QK^T and over k for PV), so no transposes are needed anywhere and
U comes out channels-major, matching the output layout directly.
"""

import ml_dtypes
import numpy as np

BF16_NP = ml_dtypes.bfloat16

import concourse.bass as bass
import concourse.mybir as mybir
import concourse.tile as tile
from concourse import bacc
from concourse.bass_utils import run_bass_kernel_spmd

HEADS = 2
D = 128          # dim head
C1 = 256         # fmap channels
C2 = 2048        # x channels
N = 4096         # spatial positions (64*64) = queries = keys
KSL = 2048       # keys per core (half)
SCALE = float(D) ** -0.5

F32 = mybir.dt.float32
BF16 = mybir.dt.bfloat16

# key-chunk = 512 keys (4 k-tiles of 128); query-chunk = 512 queries
KC = 4           # key chunks per core
KT = 4           # k-tiles (128) per key chunk
QC = 8           # query chunks of 512
QW = 512

_COMPILED = {}


def _build_program():
    nc = bacc.Bacc("TRN2", target_bir_lowering=False, debug=False, num_devices=8)

    # ---- DRAM parameters (per-core data, same program on all 8 cores) ----
    d_fmap = nc.dram_tensor("fmap_b", [2, 128, N], BF16, kind="ExternalInput").ap()
    d_fmapk = nc.dram_tensor("fmap_k", [2, 128, KSL], BF16, kind="ExternalInput").ap()
    d_xs = nc.dram_tensor("xs", [16, 128, KSL], BF16, kind="ExternalInput").ap()
    d_wqT = nc.dram_tensor("wqT", [2, 128, D], BF16, kind="ExternalInput").ap()
    d_wk1T = nc.dram_tensor("wk1T", [2, 128, D], BF16, kind="ExternalInput").ap()
    d_wvT = nc.dram_tensor("wvT", [2, 128, D], BF16, kind="ExternalInput").ap()
    d_wk2T = nc.dram_tensor("wk2T", [16, 128, D], BF16, kind="ExternalInput").ap()
    d_outU = nc.dram_tensor("outU", [128, N], F32, kind="ExternalOutput").ap()
    d_den = nc.dram_tensor("denom", [1, N], F32, kind="ExternalOutput").ap()

    with tile.TileContext(nc) as tc:
        with (
            tc.tile_pool(name="wts", bufs=1) as wts,
            tc.tile_pool(name="fm", bufs=1) as fm,
            tc.tile_pool(name="big", bufs=1) as big,
            tc.tile_pool(name="xs", bufs=2) as xsp,
            tc.tile_pool(name="ex", bufs=3) as exp_pool,
            tc.tile_pool(name="st", bufs=2) as st,
            tc.tile_pool(name="ps_k", bufs=1, space="PSUM") as ps_k,
            tc.tile_pool(name="ps_s", bufs=3, space="PSUM") as ps_s,
            tc.tile_pool(name="ps_o", bufs=2, space="PSUM") as ps_o,
            tc.tile_pool(name="ps_d", bufs=2, space="PSUM") as ps_d,
        ):
            # ---- load weights + fmap ----
            wqT = [wts.tile([128, D], BF16, tag=f"wqT{t}", name=f"wqT{t}") for t in range(2)]
            wk1T = [wts.tile([128, D], BF16, tag=f"wk1T{t}", name=f"wk1T{t}") for t in range(2)]
            wvT = [wts.tile([128, D], BF16, tag=f"wvT{t}", name=f"wvT{t}") for t in range(2)]
            wk2T = [wts.tile([128, D], BF16, tag=f"wk2T{t}", name=f"wk2T{t}") for t in range(16)]
            for t in range(2):
                nc.sync.dma_start(wqT[t][:], d_wqT[t])
                nc.sync.dma_start(wk1T[t][:], d_wk1T[t])
                nc.sync.dma_start(wvT[t][:], d_wvT[t])
            for t in range(16):
                nc.sync.dma_start(wk2T[t][:], d_wk2T[t])

            fmap = [fm.tile([128, N], BF16, tag=f"fmap{t}", name=f"fmap{t}") for t in range(2)]
            fmapk = [fm.tile([128, KSL], BF16, tag=f"fmapk{t}", name=f"fmapk{t}") for t in range(2)]
            for t in range(2):
                nc.sync.dma_start(fmap[t][:], d_fmap[t])
                nc.sync.dma_start(fmapk[t][:], d_fmapk[t])

            ones = wts.tile([128, 1], BF16, tag="ones")
            nc.vector.memset(ones[:], 1.0)

            # x chunk tiles: per (chunk parity, ct) slots via bufs=2 on tag ct
            def load_x_chunk(kc):
                tiles = []
                for ct in range(16):
                    xt = xsp.tile([128, QW], BF16, tag=f"x{ct}", name=f"x{ct}")
                    nc.sync.dma_start(xt[:], d_xs[ct][:, kc * QW:(kc + 1) * QW])
                    tiles.append(xt)
                return tiles

            x_tiles = load_x_chunk(0)

            # ---- q = Wq @ fmap  -> q_sb [d=128, N] ----
            q_sb = big.tile([128, N], BF16, tag="q")
            for nch in range(8):
                ps = ps_s.tile([128, QW], F32, tag="ps_sim")
                sl = slice(nch * QW, (nch + 1) * QW)
                nc.tensor.matmul(ps[:], wqT[0][:], fmap[0][:, sl], start=True, stop=False)
                nc.tensor.matmul(ps[:], wqT[1][:], fmap[1][:, sl], start=False, stop=True)
                nc.vector.tensor_copy(q_sb[:, sl], ps[:])

            # ---- vT tiles [k=128, d] via fmap_k-stationary matmuls ----
            vT = big.tile([128, 16, D], BF16, tag="vT")
            for kt in range(16):
                ps = ps_s.tile([128, D], F32, tag="ps_sim")
                ksl = slice(kt * 128, (kt + 1) * 128)
                nc.tensor.matmul(ps[:], fmapk[0][:, ksl], wvT[0][:], start=True, stop=False)
                nc.tensor.matmul(ps[:], fmapk[1][:, ksl], wvT[1][:], start=False, stop=True)
                nc.scalar.copy(vT[:, kt, :], ps[:])

            ksum = big.tile([128, KSL], BF16, tag="ksum")
            outU = big.tile([128, N], F32, tag="outU")
            den = big.tile([1, N], F32, tag="den")

            # ---- main loop over key chunks ----
            for kc in range(KC):
                # ksum[:, kc] = Wk1 @ fmap_k[:, kc] + Wk2 @ xs[:, kc]
                kps = ps_k.tile([128, QW], F32, tag="ps_ksum")
                sl = slice(kc * QW, (kc + 1) * QW)
                nc.tensor.matmul(kps[:], wk1T[0][:], fmapk[0][:, sl], start=True, stop=False)
                nc.tensor.matmul(kps[:], wk1T[1][:], fmapk[1][:, sl], start=False, stop=False)
                for ct in range(16):
                    nc.tensor.matmul(kps[:], wk2T[ct][:], x_tiles[ct][:],
                                     start=False, stop=(ct == 15))
                nc.vector.tensor_copy(ksum[:, sl], kps[:])

                if kc + 1 < KC:
                    x_tiles = load_x_chunk(kc + 1)

                # attention over this chunk's 4 k-tiles, all 8 query chunks
                for qc in range(QC):
                    qsl = slice(qc * QW, (qc + 1) * QW)
                    ops = ps_o.tile([128, QW], F32, tag="ps_out")
                    dps = ps_d.tile([1, QW], F32, tag="ps_den")
                    for kt in range(KT):
                        kk = kc * KT + kt
                        sps = ps_s.tile([128, QW], F32, tag="ps_sim")
                        nc.tensor.matmul(
                            sps[:], ksum[:, kk * 128:(kk + 1) * 128], q_sb[:, qsl],
                            start=True, stop=True)
                        et = exp_pool.tile([128, QW], BF16, tag="exp")
                        nc.scalar.activation(et[:], sps[:],
                                             mybir.ActivationFunctionType.Exp,
                                             scale=SCALE)
                        nc.tensor.matmul(ops[:], vT[:, kk, :], et[:],
                                         start=(kt == 0), stop=(kt == KT - 1))
                        nc.tensor.matmul(dps[:], ones[:], et[:],
                                         start=(kt == 0), stop=(kt == KT - 1))
                    if kc == 0:
                        nc.vector.tensor_copy(outU[:, qsl], ops[:])
                        nc.vector.tensor_copy(den[:, qsl], dps[:])
                    else:
                        nc.vector.tensor_add(outU[:, qsl], outU[:, qsl], ops[:])
                        nc.vector.tensor_add(den[:, qsl], den[:, qsl], dps[:])
                        if kc == KC - 1:
                            st_t = st.tile([128, QW], F32, tag="st")
                            nc.vector.tensor_copy(st_t[:], outU[:, qsl])
                            nc.sync.dma_start(d_outU[:, qsl], st_t[:])
            nc.sync.dma_start(d_den[:], den[:])

    nc.compile()
    return nc


def _prep_inputs(fmap, x, Wqkv, Wk2):
    """Host-side slicing: per-core input dicts. Core c = b*4 + h*2 + kh."""
    fmap = np.ascontiguousarray(fmap, dtype=np.float32)
    x = np.ascontiguousarray(x, dtype=np.float32)
    Wqkv = np.ascontiguousarray(Wqkv, dtype=np.float32)
    Wk2 = np.ascontiguousarray(Wk2, dtype=np.float32)

    in_maps = []
    for c in range(8):
        b, h, kh = c // 4, (c // 2) % 2, c % 2
        fb = fmap[b].reshape(C1, N)
        xb = x[b].reshape(C2, N)
        ks = slice(kh * KSL, (kh + 1) * KSL)
        wq = Wqkv[h * D:(h + 1) * D]              # [128, 256]
        wk1 = Wqkv[C1 + h * D:C1 + (h + 1) * D]
        wv = Wqkv[2 * C1 + h * D:2 * C1 + (h + 1) * D]
        wk2 = Wk2[h * D:(h + 1) * D]              # [128, 2048]
        in_maps.append({
            "fmap_b": fb.reshape(2, 128, N).astype(BF16_NP),
            "fmap_k": np.ascontiguousarray(fb[:, ks].reshape(2, 128, KSL)).astype(BF16_NP),
            "xs": np.ascontiguousarray(xb[:, ks].reshape(16, 128, KSL)).astype(BF16_NP),
            "wqT": np.ascontiguousarray(wq.T).reshape(2, 128, D).astype(BF16_NP),
            "wk1T": np.ascontiguousarray(wk1.T).reshape(2, 128, D).astype(BF16_NP),
            "wvT": np.ascontiguousarray(wv.T).reshape(2, 128, D).astype(BF16_NP),
            "wk2T": np.ascontiguousarray(wk2.T).reshape(16, 128, D).astype(BF16_NP),
        })
    return in_maps


def _combine(results):
    """Host epilogue: add key-half partials, normalize, assemble output."""
    out = np.empty((2, HEADS * D, 64, 64), dtype=np.float32)
    for b in range(2):
        for h in range(2):
            c0 = b * 4 + h * 2
            U = results[c0]["outU"] + results[c0 + 1]["outU"]     # [128, N]
            Dn = results[c0]["denom"] + results[c0 + 1]["denom"]  # [1, N]
            out[b, h * D:(h + 1) * D] = (U / Dn).reshape(D, 64, 64)
    return out


def run_on_device(in_maps, trace=False, **kw):
    if "nc" not in _COMPILED:
        _COMPILED["nc"] = _build_program()
    return run_bass_kernel_spmd(_COMPILED["nc"], in_maps, list(range(8)),
                                trace=trace, **kw)


def kernel(fmap, x, Wqkv, Wk2):
    in_maps = _prep_inputs(fmap, x, Wqkv, Wk2)
    res = run_on_device(in_maps)
    return _combine(res.results)
